# revision 44
# baseline (speedup 1.0000x reference)
"""Block-sparse linear kernel for Trainium2 (8 NeuronCores, SPMD).

Computes out = x @ W.T + bias where W is a 4096x4096 block-sparse matrix
given as 8192 active 32x32 blocks (50% density).

Strategy:
  - Data-parallel over tokens: 8192 tokens -> 1024 per core; weights replicated.
  - On device, compute out.T = W @ x.T with dense TensorE matmuls
    (the 32x32 random sparsity cannot beat the dense array roofline on TRN2:
    sub-array packed matmuls are weight-load-port bound at ~34ns/block,
    2x worse than the dense stream), accumulate in fp32 PSUM, fused bias
    add on psum evacuation, DMA out.
  - Mixed precision: P k-tile-pairs (2P of 32 k-tiles) run as fp8e4
    DoubleRow matmuls (K=256 per MM at the same 216ns/MM as a K=128 fp16
    MM -> true 2x); the remaining k-tiles run fp16. P=4 (alpha=0.25)
    measures rel_norm 1.6e-2 on the seed-0 data (gate 2e-2).
  - Everything scaled by S=64 on host (W*64, bias*64) so fp8 W avoids
    e4m3 subnormals; host divides the output by 64.
  - Host densifies/pre-transposes weights into SBUF-image layout and
    transposes x/out (cheap numpy work, off the device critical path).
"""

import os
import numpy as np
import ml_dtypes

import concourse.bacc as bacc
import concourse.mybir as mybir
import concourse.tile as tile
from concourse.bass_utils import run_bass_kernel_spmd

SLIM_TAIL = os.environ.get("KERNEL_SLIM_TAIL", "1") == "1"
if SLIM_TAIL:
    from concourse.vector_clock import ScopedClock as _ScopedClock

    def _slim_drain_and_barrier(self, tick_clock, wait_clock):
        # Same as TileContext._drain_and_barrier but without the trailing
        # all-engine barrier: each engine's sem clears are ordered before
        # NEFF completion by its own program order, so re-execution still
        # sees cleared semaphores. Saves ~3.5us of kernel tail.
        drain_inst = self.nc.sync.drain()
        wait_clock.add_sem_waits(
            drain_inst.ins, _ScopedClock({None: tick_clock.global_clock})
        )
        self.nc.all_engine_barrier()
        popped = self.nc._tile_sem_poison_stack.pop()
        assert popped is self._sem_poison
        self.nc.clear_and_free_semaphores(list(self.sems.allocated().values()))

    tile.TileContext._drain_and_barrier = _slim_drain_and_barrier

TOKENS = 8192
IN = 4096
OUT = 4096
BS = 32
NBR = OUT // BS   # 128 block rows
NBC = IN // BS    # 128 block cols
NCORES = 8
TPC = TOKENS // NCORES   # 1024 tokens per core

MCH = 128   # output chunk (psum partitions)
KCH = 128   # contraction chunk (sbuf partitions)
NCH = 512   # token chunk (psum free dim, one bank of fp32)
NM = OUT // MCH    # 32
NK = IN // KCH     # 32 total k-tiles
NN = TPC // NCH    # 2

P8 = int(os.environ.get("KERNEL_P8", "4"))     # fp8 DoubleRow k-tile PAIRS
NK16 = NK - 2 * P8                             # fp16 k-tiles
SCALE = 64.0

WBUFS = int(os.environ.get("KERNEL_WBUFS", "6"))
PSUM_BUFS = int(os.environ.get("KERNEL_PSUM_BUFS", "7"))
WARM_MMS = int(os.environ.get("KERNEL_WARM_MMS", "100"))
NPIN = int(os.environ.get("KERNEL_NPIN", "8"))   # m-tiles with W pinned, n1 deferred
XSPLIT = int(os.environ.get("KERNEL_XSPLIT", "3"))   # x16 n0 quad-tiles on Q7
HEADM_DEF = 6   # groups whose fp8 DR blocks run before any fp16 MM
TAILSPLIT = int(os.environ.get("KERNEL_TAILSPLIT", "2"))  # groups w/ chunked drain

_CACHE: dict = {}

F16 = mybir.dt.float16
FP8 = mybir.dt.float8e4
F32 = mybir.dt.float32


def _build():
    """Mixed fp8-DoubleRow / fp16 matmul module:
    out.T[m] = sum_kp W8[kp,m].T @ x8.T[kp] + sum_k W16[k,m].T @ x16.T[k],
    all scaled by 64; bias64 added on psum evacuation."""
    nc = bacc.Bacc("TRN2", target_bir_lowering=False, debug=False)

    NQ16 = NK16 // 4   # quad-k x tiles (4KB DMA lines)
    wt16 = nc.dram_tensor("wt16", [NM, KCH, NK16 * MCH], F16,
                          kind="ExternalInput")
    xt16 = nc.dram_tensor("xt16", [NQ16, KCH, 4 * NCH], F16,
                          kind="ExternalInput")
    # n=1 x in one partition-major slab: a single 3MB DMA (24KB lines) that
    # lands by ~+36us, long before the first n=1 group (~+61us). One tile
    # instead of six = 5 fewer semaphores in the serialized teardown.
    xt16n1 = nc.dram_tensor("xt16n1", [KCH, NQ16 * 4 * NCH], F16,
                            kind="ExternalInput")
    if P8:
        wt8 = nc.dram_tensor("wt8", [NM, KCH, P8, 2, MCH], FP8,
                             kind="ExternalInput")
        # Head w8s in partition-major layout: one DMA with 6KB lines lands
        # in ~5us, vs ~7us for six 1KB-line per-m transfers that also queue
        # ahead of w16(m0). A stall here crosses the 3.4us HAM window and
        # the next ~13 MMs run at 1.2GHz.
        wt8h = nc.dram_tensor("wt8h", [KCH, HEADM_DEF * P8 * 2 * MCH], FP8,
                              kind="ExternalInput")
        xt8 = nc.dram_tensor("xt8", [NN, KCH, P8 * 2 * NCH], FP8,
                             kind="ExternalInput")
    bias_img = nc.dram_tensor("bias_img", [MCH, NM], F32, kind="ExternalInput")
    outT = nc.dram_tensor("outT", [NM, MCH, TPC], F32, kind="ExternalOutput")

    DR = mybir.MatmulPerfMode.DoubleRow

    with tile.TileContext(nc) as tc:
        with (
            tc.tile_pool(name="xres", bufs=NK16 // 4) as xres,
            tc.tile_pool(name="xres8", bufs=NN) as xres8,
            tc.tile_pool(name="wpin", bufs=max(NPIN, 1)) as wpin,
            tc.tile_pool(name="wpin8", bufs=max(NPIN - HEADM_DEF, 1)) as wpin8,
            tc.tile_pool(name="wbuf", bufs=WBUFS) as wbuf,
            tc.tile_pool(name="wbuf8", bufs=3) as wbuf8,
            tc.tile_pool(name="obuf", bufs=3) as obuf,
            tc.tile_pool(name="misc", bufs=1) as misc,
            tc.tile_pool(name="ps", bufs=PSUM_BUFS, space="PSUM") as ps,
        ):
            HEADM = min(HEADM_DEF, NPIN) if P8 else 0
            if P8:
                w8head = misc.tile([KCH, HEADM_DEF, P8, 2, MCH], FP8,
                                   tag="w8h")
                nc.sync.dma_start(w8head[:], wt8h.ap())
            bias_t = misc.tile([MCH, NM], F32, tag="bias")
            nc.sync.dma_start(bias_t[:], bias_img.ap())

            # PE warm-up: the HAM clock gate keeps the array at 1.2 GHz until
            # ~3.4us of sustained activity. Run throwaway matmuls on a local
            # zeroed tile during the initial DMA wait so real matmuls start
            # at 2.4 GHz and the PE never sits idle past a MID window.
            if WARM_MMS:
                # The source tile must be written before the PE reads it:
                # matmul on never-written SBUF wedges the device (parity).
                # DVE memset (not gpsimd): the Q7 takes ~6us to start, which
                # would delay the whole warmup chain.
                wz = misc.tile([KCH, MCH], F16, tag="wz")
                nc.vector.memset(wz[:], 0.0)
                pwarm = ps.tile([MCH, 64], F32, tag="pw",
                                name="pwarm", bufs=1)
                for _ in range(WARM_MMS):
                    nc.tensor.matmul(pwarm[:], wz[:], wz[:, :64],
                                     start=True, stop=True)

            # x on the ACT HWDGE ring, n-major so the n=0 sweep's data lands
            # first; W/out use the SP ring. DRAM layout is quad-k-merged so
            # DMA lines are 4KB (the ACT queue ran at only ~117GB/s on 1KB
            # lines — descriptor-rate limited). fp8 x first (it gates the DR
            # matmuls that open each psum group). The last XSPLIT n=0 quad
            # tiles can go on the Q7 SWDGE queue for extra head bandwidth.
            xq16 = {}
            x8n = {}
            for n in range(NN):
                if P8:
                    t8 = xres8.tile([KCH, P8, 2, NCH], FP8, tag="x8",
                                    name=f"x8_{n}")
                    nc.scalar.dma_start(t8[:], xt8.ap()[n])
                    x8n[n] = t8
                if n == 0:
                    for q in range(NQ16):
                        t = xres.tile([KCH, 4 * NCH], F16, tag="x",
                                      name=f"x{q}_0")
                        eng = nc.gpsimd if q >= NQ16 - XSPLIT else nc.scalar
                        eng.dma_start(t[:], xt16.ap()[q])
                        xq16[q] = t
                else:
                    x16n1 = xres.tile([KCH, NQ16, 4 * NCH], F16, tag="xn1",
                                      name="x16n1", bufs=1)
                    nc.scalar.dma_start(x16n1[:], xt16n1.ap())

            def x16ap(k, n):
                if n == 0:
                    return xq16[k // 4][:, (k % 4) * NCH:(k % 4 + 1) * NCH]
                return x16n1[:, k // 4, (k % 4) * NCH:(k % 4 + 1) * NCH]

            def x8ap(kp, n):
                return x8n[n][:, kp]

            # m-outer, n-inner: W streamed ONCE per m-tile (halves the sync
            # ring load to ~120GB/s — at n-outer's 2x W the ring ran at its
            # ~190GB/s capacity, backing up the tail). The first NPIN
            # m-tiles' n=1 groups are deferred to the end (their W stays
            # pinned in SBUF) so no group needs x n=1 before it lands.
            schedule = [(m, 0) for m in range(NPIN)]
            for m in range(NPIN, NM):
                schedule += [(m, 0), (m, 1)]
            schedule += [(m, 1) for m in range(NPIN)]

            # Head: the first HEADM groups' fp8 DR blocks run before any
            # fp16 MM. Their inputs (the merged w8head, fp8 x) land within
            # ~8us, while the fp16 x quads trickle in until ~15us — this
            # gives the PE ~5us of real work to chew during that wait
            # (program order on the PE queue is otherwise head-of-line
            # blocked by group 0's fp16 MMs). 6 open groups + pwarm + 1
            # rotating spare = exactly the 8 PSUM banks.
            w8tiles = {}
            wtiles = {}

            def w8ap(m, kp):
                if m < HEADM:
                    return w8head[:, m, kp]
                return w8tiles[m][:, kp]

            def load_w(m):
                if m in wtiles:
                    return
                if P8 and m >= HEADM and m not in w8tiles:
                    wp8 = wpin8 if m < NPIN else wbuf8
                    w8 = wp8.tile([KCH, P8, 2, MCH], FP8, tag="w8",
                                  name=f"w8_{m}")
                    nc.sync.dma_start(w8[:], wt8.ap()[m])
                    w8tiles[m] = w8
                wp = wpin if m < NPIN else wbuf
                w = wp.tile([KCH, NK16 * MCH], F16, tag="w", name=f"w{m}")
                for c in range(4):
                    cs = c * (NK16 * MCH // 4)
                    ce = (c + 1) * (NK16 * MCH // 4)
                    nc.sync.dma_start(w[:, cs:ce], wt16.ap()[m][:, cs:ce])
                wtiles[m] = w

            head_ps = {}
            for m in range(HEADM):
                load_w(m)
            for m in range(HEADM):
                p = ps.tile([MCH, NCH], F32, tag="p", name=f"p0_{m}")
                for kp in range(P8):
                    nc.tensor.matmul(
                        p[:], w8ap(m, kp), x8ap(kp, 0),
                        start=(kp == 0), stop=False, perf_mode=DR)
                head_ps[m] = p

            for gi, (m, n) in enumerate(schedule):
                load_w(m)
                w = wtiles[m]
                if n == 0 and m in head_ps:
                    p = head_ps[m]
                else:
                    p = ps.tile([MCH, NCH], F32, tag="p", name=f"p{n}_{m}")
                    for kp in range(P8):
                        nc.tensor.matmul(
                            p[:], w8ap(m, kp), x8ap(kp, n),
                            start=(kp == 0), stop=False, perf_mode=DR)
                for k in range(NK16):
                    nc.tensor.matmul(
                        p[:], w[:, k * MCH:(k + 1) * MCH], x16ap(k, n),
                        start=(k == 0 and P8 == 0), stop=(k == NK16 - 1))
                o = obuf.tile([MCH, NCH], F32, tag="o", name=f"o{n}_{m}")
                if gi >= len(schedule) - TAILSPLIT:
                    # Chunked drain so the final out DMA starts before
                    # the whole psum evacuation finishes (tail shave).
                    CH = NCH // 4
                    for c in range(4):
                        sl = slice(c * CH, (c + 1) * CH)
                        nc.vector.tensor_scalar_add(
                            o[:, sl], p[:, sl], bias_t[:, m:m + 1])
                        nc.sync.dma_start(
                            outT.ap()[m][:, n * NCH + c * CH:
                                         n * NCH + (c + 1) * CH],
                            o[:, sl])
                else:
                    nc.vector.tensor_scalar_add(o[:], p[:],
                                                bias_t[:, m:m + 1])
                    nc.sync.dma_start(
                        outT.ap()[m][:, n * NCH:(n + 1) * NCH], o[:])

    nc.compile()
    return nc


def _get_nc():
    if "nc" not in _CACHE:
        _CACHE["nc"] = _build()
    return _CACHE["nc"]


def _densify(weight_data, block_rows, block_cols):
    """Scatter 32x32 blocks into dense W (OUT, IN)."""
    w4 = np.zeros((NBR, NBC, BS, BS), dtype=np.float32)
    w4[block_rows, block_cols] = weight_data
    return w4.transpose(0, 2, 1, 3).reshape(OUT, IN)


def _make_in_maps(x, weight_data, bias, block_rows, block_cols):
    W = _densify(np.asarray(weight_data, dtype=np.float32),
                 np.asarray(block_rows), np.asarray(block_cols)) * SCALE
    x = np.asarray(x, dtype=np.float32)
    KF8 = 2 * P8 * KCH   # fp8 k-range (features 0..KF8)

    # fp8 part: wt8[m][i][kp][t][o] = W[m*128+o, (2kp+t)*128+i]
    W8 = W[:, :KF8].astype(ml_dtypes.float8_e4m3)
    wt8 = np.ascontiguousarray(
        W8.reshape(NM, MCH, P8, 2, KCH).transpose(0, 4, 2, 3, 1))
    # head w8s, partition-major: wt8h[i][m*P8*2*128 + ...] (6KB DMA lines)
    wt8h = np.ascontiguousarray(
        wt8[:HEADM_DEF].transpose(1, 0, 2, 3, 4)).reshape(KCH, -1)
    # xt8[core][n][i][kp*2+t][nn] = x[core*TPC+n*NCH+nn, (2kp+t)*128+i]
    x8 = x[:, :KF8].astype(ml_dtypes.float8_e4m3)
    xt8_all = np.ascontiguousarray(
        x8.reshape(NCORES, NN, NCH, P8, 2, KCH).transpose(0, 1, 5, 3, 4, 2)
    ).reshape(NCORES, NN, KCH, P8 * 2 * NCH)

    # fp16 part: wt16[m][i2, k*128+o2] = W[m*128+o2, KF8 + k*128+i2]
    W16 = W[:, KF8:].astype(np.float16)
    wt16 = np.ascontiguousarray(
        W16.reshape(NM, MCH, NK16, KCH).transpose(0, 3, 2, 1)
    ).reshape(NM, KCH, NK16 * MCH)
    # xt16[core][q][i2][j*NCH+t] = x[core*TPC+t, KF8+(4q+j)*128+i2]  (n=0)
    # xt16n1[core][i2][q*4*NCH+j*NCH+t] = same for the n=1 token half
    NQ16 = NK16 // 4
    x16v = x[:, KF8:].reshape(NCORES, NN, NCH, NQ16, 4, KCH)
    xt16_all = np.ascontiguousarray(
        x16v[:, 0].transpose(0, 2, 4, 3, 1).astype(np.float16)
    ).reshape(NCORES, NQ16, KCH, 4 * NCH)
    xt16n1_all = np.ascontiguousarray(
        x16v[:, 1].transpose(0, 4, 2, 3, 1).astype(np.float16)
    ).reshape(NCORES, KCH, NQ16 * 4 * NCH)
    bias_img = np.ascontiguousarray(
        (np.asarray(bias, dtype=np.float32) * SCALE).reshape(NM, MCH).T
    )
    maps = []
    for c in range(NCORES):
        m = {"wt16": wt16, "xt16": xt16_all[c], "xt16n1": xt16n1_all[c],
             "bias_img": bias_img}
        if P8:
            m["wt8"] = wt8
            m["wt8h"] = wt8h
            m["xt8"] = xt8_all[c]
        maps.append(m)
    return maps


def _assemble(results):
    out = np.empty((TOKENS, OUT), dtype=np.float32)
    inv = np.float32(1.0 / SCALE)
    for c, r in enumerate(results):
        out[c * TPC:(c + 1) * TPC] = r["outT"].reshape(OUT, TPC).T * inv
    return out


def kernel(x, weight_data, bias, block_rows, block_cols):
    nc = _get_nc()
    in_maps = _make_in_maps(x, weight_data, bias, block_rows, block_cols)
    res = run_bass_kernel_spmd(nc, in_maps, core_ids=list(range(NCORES)))
    return _assemble(res.results)


# revision 50
# speedup vs baseline: 1.0259x; 1.0259x over previous
"""Block-sparse linear kernel for Trainium2 (8 NeuronCores, SPMD).

Computes out = x @ W.T + bias where W is a 4096x4096 block-sparse matrix
given as 8192 active 32x32 blocks (50% density).

Strategy:
  - Data-parallel over tokens: 8192 tokens -> 1024 per core; weights replicated.
  - On device, compute out.T = W @ x.T with dense TensorE matmuls
    (the 32x32 random sparsity cannot beat the dense array roofline on TRN2:
    sub-array packed matmuls are weight-load-port bound at ~34ns/block,
    2x worse than the dense stream), accumulate in fp32 PSUM, fused bias
    add on psum evacuation, DMA out.
  - Mixed precision: P k-tile-pairs (2P of 32 k-tiles) run as fp8e4
    DoubleRow matmuls (K=256 per MM at the same 216ns/MM as a K=128 fp16
    MM -> true 2x); the remaining k-tiles run fp16. P=4 (alpha=0.25)
    measures rel_norm 1.6e-2 on the seed-0 data (gate 2e-2).
  - Everything scaled by S=64 on host (W*64, bias*64) so fp8 W avoids
    e4m3 subnormals; host divides the output by 64.
  - Host densifies/pre-transposes weights into SBUF-image layout and
    transposes x/out (cheap numpy work, off the device critical path).
"""

import os
import numpy as np
import ml_dtypes

import concourse.bacc as bacc
import concourse.mybir as mybir
import concourse.tile as tile
from concourse.bass_utils import run_bass_kernel_spmd

SLIM_TAIL = os.environ.get("KERNEL_SLIM_TAIL", "1") == "1"
if SLIM_TAIL:
    from concourse.vector_clock import ScopedClock as _ScopedClock

    def _slim_drain_and_barrier(self, tick_clock, wait_clock):
        # Same as TileContext._drain_and_barrier but without the trailing
        # all-engine barrier: each engine's sem clears are ordered before
        # NEFF completion by its own program order, so re-execution still
        # sees cleared semaphores. Saves ~3.5us of kernel tail.
        drain_inst = self.nc.sync.drain()
        wait_clock.add_sem_waits(
            drain_inst.ins, _ScopedClock({None: tick_clock.global_clock})
        )
        self.nc.all_engine_barrier()
        popped = self.nc._tile_sem_poison_stack.pop()
        assert popped is self._sem_poison
        self.nc.clear_and_free_semaphores(list(self.sems.allocated().values()))

    tile.TileContext._drain_and_barrier = _slim_drain_and_barrier

TOKENS = 8192
IN = 4096
OUT = 4096
BS = 32
NBR = OUT // BS   # 128 block rows
NBC = IN // BS    # 128 block cols
NCORES = 8
TPC = TOKENS // NCORES   # 1024 tokens per core

MCH = 128   # output chunk (psum partitions)
KCH = 128   # contraction chunk (sbuf partitions)
NCH = 512   # token chunk (psum free dim, one bank of fp32)
NM = OUT // MCH    # 32
NK = IN // KCH     # 32 total k-tiles
NN = TPC // NCH    # 2

P8 = int(os.environ.get("KERNEL_P8", "4"))     # fp8 DoubleRow k-tile PAIRS
NK16 = NK - 2 * P8                             # fp16 k-tiles
SCALE = 64.0

WBUFS = int(os.environ.get("KERNEL_WBUFS", "6"))
PSUM_BUFS = int(os.environ.get("KERNEL_PSUM_BUFS", "7"))
WARM_MMS = int(os.environ.get("KERNEL_WARM_MMS", "80"))
NPIN = int(os.environ.get("KERNEL_NPIN", "8"))   # m-tiles with W pinned, n1 deferred
XSPLIT = int(os.environ.get("KERNEL_XSPLIT", "3"))   # x16 n0 quad-tiles on Q7
HEADM_DEF = 6   # groups whose fp8 DR blocks run before any fp16 MM
TAILSPLIT = int(os.environ.get("KERNEL_TAILSPLIT", "2"))  # groups w/ chunked drain

_CACHE: dict = {}

F16 = mybir.dt.float16
FP8 = mybir.dt.float8e4
F32 = mybir.dt.float32


def _build():
    """Mixed fp8-DoubleRow / fp16 matmul module:
    out.T[m] = sum_kp W8[kp,m].T @ x8.T[kp] + sum_k W16[k,m].T @ x16.T[k],
    all scaled by 64; bias64 added on psum evacuation."""
    nc = bacc.Bacc("TRN2", target_bir_lowering=False, debug=False)

    NQ16 = NK16 // 4   # quad-k x tiles (4KB DMA lines)
    wt16 = nc.dram_tensor("wt16", [NM, KCH, NK16 * MCH], F16,
                          kind="ExternalInput")
    xt16 = nc.dram_tensor("xt16", [NN, NQ16, KCH, 4 * NCH], F16,
                          kind="ExternalInput")
    if P8:
        wt8 = nc.dram_tensor("wt8", [NM, KCH, P8, 2, MCH], FP8,
                             kind="ExternalInput")
        # Head w8s in partition-major layout: one DMA with 6KB lines lands
        # in ~5us, vs ~7us for six 1KB-line per-m transfers that also queue
        # ahead of w16(m0). A stall here crosses the 3.4us HAM window and
        # the next ~13 MMs run at 1.2GHz.
        wt8h = nc.dram_tensor("wt8h", [KCH, HEADM_DEF * P8 * 2 * MCH], FP8,
                              kind="ExternalInput")
        xt8 = nc.dram_tensor("xt8", [NN, KCH, P8 * 2 * NCH], FP8,
                             kind="ExternalInput")
    bias_img = nc.dram_tensor("bias_img", [MCH, NM], F32, kind="ExternalInput")
    outT = nc.dram_tensor("outT", [NM, MCH, TPC], F32, kind="ExternalOutput")

    DR = mybir.MatmulPerfMode.DoubleRow

    with tile.TileContext(nc) as tc:
        with (
            tc.tile_pool(name="xres", bufs=(NK16 // 4) * NN) as xres,
            tc.tile_pool(name="xres8", bufs=NN) as xres8,
            tc.tile_pool(name="wpin", bufs=max(NPIN, 1)) as wpin,
            tc.tile_pool(name="wpin8", bufs=max(NPIN - HEADM_DEF, 1)) as wpin8,
            tc.tile_pool(name="wbuf", bufs=WBUFS) as wbuf,
            tc.tile_pool(name="wbuf8", bufs=3) as wbuf8,
            tc.tile_pool(name="obuf", bufs=3) as obuf,
            tc.tile_pool(name="misc", bufs=1) as misc,
            tc.tile_pool(name="ps", bufs=PSUM_BUFS, space="PSUM") as ps,
        ):
            HEADM = min(HEADM_DEF, NPIN) if P8 else 0
            if P8:
                w8head = misc.tile([KCH, HEADM_DEF, P8, 2, MCH], FP8,
                                   tag="w8h")
                nc.sync.dma_start(w8head[:], wt8h.ap())
            bias_t = misc.tile([MCH, NM], F32, tag="bias")
            nc.sync.dma_start(bias_t[:], bias_img.ap())

            # PE warm-up: the HAM clock gate keeps the array at 1.2 GHz until
            # ~3.4us of sustained activity. Run throwaway matmuls on a local
            # zeroed tile during the initial DMA wait so real matmuls start
            # at 2.4 GHz and the PE never sits idle past a MID window.
            if WARM_MMS:
                # The source tile must be written before the PE reads it:
                # matmul on never-written SBUF wedges the device (parity).
                # DVE memset (not gpsimd): the Q7 takes ~6us to start, which
                # would delay the whole warmup chain.
                wz = misc.tile([KCH, MCH], F16, tag="wz")
                nc.vector.memset(wz[:], 0.0)
                pwarm = ps.tile([MCH, 64], F32, tag="pw",
                                name="pwarm", bufs=1)
                for _ in range(WARM_MMS):
                    nc.tensor.matmul(pwarm[:], wz[:], wz[:, :64],
                                     start=True, stop=True)

            # x on the ACT HWDGE ring, n-major so the n=0 sweep's data lands
            # first; W/out use the SP ring. DRAM layout is quad-k-merged so
            # DMA lines are 4KB (the ACT queue ran at only ~117GB/s on 1KB
            # lines — descriptor-rate limited). fp8 x first (it gates the DR
            # matmuls that open each psum group). The last XSPLIT n=0 quad
            # tiles can go on the Q7 SWDGE queue for extra head bandwidth.
            xq16 = {}
            x8n = {}
            for n in range(NN):
                if P8:
                    t8 = xres8.tile([KCH, P8, 2, NCH], FP8, tag="x8",
                                    name=f"x8_{n}")
                    nc.scalar.dma_start(t8[:], xt8.ap()[n])
                    x8n[n] = t8
                for q in range(NQ16):
                    t = xres.tile([KCH, 4 * NCH], F16, tag="x", name=f"x{q}_{n}")
                    eng = nc.gpsimd if (n == 0 and q >= NQ16 - XSPLIT) \
                        else nc.scalar
                    eng.dma_start(t[:], xt16.ap()[n][q])
                    xq16[(q, n)] = t

            def x16ap(k, n):
                return xq16[(k // 4, n)][:, (k % 4) * NCH:(k % 4 + 1) * NCH]

            def x8ap(kp, n):
                return x8n[n][:, kp]

            # m-outer, n-inner: W streamed ONCE per m-tile (halves the sync
            # ring load to ~120GB/s — at n-outer's 2x W the ring ran at its
            # ~190GB/s capacity, backing up the tail). The first NPIN
            # m-tiles' n=1 groups are deferred to the end (their W stays
            # pinned in SBUF) so no group needs x n=1 before it lands.
            schedule = [(m, 0) for m in range(NPIN)]
            for m in range(NPIN, NM):
                schedule += [(m, 0), (m, 1)]
            schedule += [(m, 1) for m in range(NPIN)]

            # Head: the first HEADM groups' fp8 DR blocks run before any
            # fp16 MM. Their inputs (the merged w8head, fp8 x) land within
            # ~8us, while the fp16 x quads trickle in until ~15us — this
            # gives the PE ~5us of real work to chew during that wait
            # (program order on the PE queue is otherwise head-of-line
            # blocked by group 0's fp16 MMs). 6 open groups + pwarm + 1
            # rotating spare = exactly the 8 PSUM banks.
            w8tiles = {}
            wtiles = {}

            def w8ap(m, kp):
                if m < HEADM:
                    return w8head[:, m, kp]
                return w8tiles[m][:, kp]

            def load_w(m):
                if m in wtiles:
                    return
                if P8 and m >= HEADM and m not in w8tiles:
                    wp8 = wpin8 if m < NPIN else wbuf8
                    w8 = wp8.tile([KCH, P8, 2, MCH], FP8, tag="w8",
                                  name=f"w8_{m}")
                    nc.sync.dma_start(w8[:], wt8.ap()[m])
                    w8tiles[m] = w8
                wp = wpin if m < NPIN else wbuf
                w = wp.tile([KCH, NK16 * MCH], F16, tag="w", name=f"w{m}")
                for c in range(4):
                    cs = c * (NK16 * MCH // 4)
                    ce = (c + 1) * (NK16 * MCH // 4)
                    nc.sync.dma_start(w[:, cs:ce], wt16.ap()[m][:, cs:ce])
                wtiles[m] = w

            head_ps = {}
            for m in range(HEADM):
                load_w(m)
            for m in range(HEADM):
                p = ps.tile([MCH, NCH], F32, tag="p", name=f"p0_{m}")
                for kp in range(P8):
                    nc.tensor.matmul(
                        p[:], w8ap(m, kp), x8ap(kp, 0),
                        start=(kp == 0), stop=False, perf_mode=DR)
                head_ps[m] = p

            for gi, (m, n) in enumerate(schedule):
                load_w(m)
                w = wtiles[m]
                if n == 0 and m in head_ps:
                    p = head_ps[m]
                else:
                    p = ps.tile([MCH, NCH], F32, tag="p", name=f"p{n}_{m}")
                    for kp in range(P8):
                        nc.tensor.matmul(
                            p[:], w8ap(m, kp), x8ap(kp, n),
                            start=(kp == 0), stop=False, perf_mode=DR)
                for k in range(NK16):
                    nc.tensor.matmul(
                        p[:], w[:, k * MCH:(k + 1) * MCH], x16ap(k, n),
                        start=(k == 0 and P8 == 0), stop=(k == NK16 - 1))
                o = obuf.tile([MCH, NCH], F32, tag="o", name=f"o{n}_{m}")
                if gi >= len(schedule) - TAILSPLIT:
                    # Chunked drain so the final out DMA starts before
                    # the whole psum evacuation finishes (tail shave).
                    CH = NCH // 4
                    for c in range(4):
                        sl = slice(c * CH, (c + 1) * CH)
                        nc.vector.tensor_scalar_add(
                            o[:, sl], p[:, sl], bias_t[:, m:m + 1])
                        nc.sync.dma_start(
                            outT.ap()[m][:, n * NCH + c * CH:
                                         n * NCH + (c + 1) * CH],
                            o[:, sl])
                else:
                    nc.vector.tensor_scalar_add(o[:], p[:],
                                                bias_t[:, m:m + 1])
                    nc.sync.dma_start(
                        outT.ap()[m][:, n * NCH:(n + 1) * NCH], o[:])

    nc.compile()
    return nc


def _get_nc():
    if "nc" not in _CACHE:
        _CACHE["nc"] = _build()
    return _CACHE["nc"]


def _densify(weight_data, block_rows, block_cols):
    """Scatter 32x32 blocks into dense W (OUT, IN)."""
    w4 = np.zeros((NBR, NBC, BS, BS), dtype=np.float32)
    w4[block_rows, block_cols] = weight_data
    return w4.transpose(0, 2, 1, 3).reshape(OUT, IN)


def _make_in_maps(x, weight_data, bias, block_rows, block_cols):
    W = _densify(np.asarray(weight_data, dtype=np.float32),
                 np.asarray(block_rows), np.asarray(block_cols)) * SCALE
    x = np.asarray(x, dtype=np.float32)
    KF8 = 2 * P8 * KCH   # fp8 k-range (features 0..KF8)

    # fp8 part: wt8[m][i][kp][t][o] = W[m*128+o, (2kp+t)*128+i]
    W8 = W[:, :KF8].astype(ml_dtypes.float8_e4m3)
    wt8 = np.ascontiguousarray(
        W8.reshape(NM, MCH, P8, 2, KCH).transpose(0, 4, 2, 3, 1))
    # head w8s, partition-major: wt8h[i][m*P8*2*128 + ...] (6KB DMA lines)
    wt8h = np.ascontiguousarray(
        wt8[:HEADM_DEF].transpose(1, 0, 2, 3, 4)).reshape(KCH, -1)
    # xt8[core][n][i][kp*2+t][nn] = x[core*TPC+n*NCH+nn, (2kp+t)*128+i]
    x8 = x[:, :KF8].astype(ml_dtypes.float8_e4m3)
    xt8_all = np.ascontiguousarray(
        x8.reshape(NCORES, NN, NCH, P8, 2, KCH).transpose(0, 1, 5, 3, 4, 2)
    ).reshape(NCORES, NN, KCH, P8 * 2 * NCH)

    # fp16 part: wt16[m][i2, k*128+o2] = W[m*128+o2, KF8 + k*128+i2]
    W16 = W[:, KF8:].astype(np.float16)
    wt16 = np.ascontiguousarray(
        W16.reshape(NM, MCH, NK16, KCH).transpose(0, 3, 2, 1)
    ).reshape(NM, KCH, NK16 * MCH)
    # xt16[core][n][q][i2][j*NCH+t] = x[core*TPC+n*NCH+t, KF8+(4q+j)*128+i2]
    NQ16 = NK16 // 4
    xt16_all = np.ascontiguousarray(
        x[:, KF8:]
        .reshape(NCORES, NN, NCH, NQ16, 4, KCH).transpose(0, 1, 3, 5, 4, 2)
        .astype(np.float16)
    ).reshape(NCORES, NN, NQ16, KCH, 4 * NCH)
    bias_img = np.ascontiguousarray(
        (np.asarray(bias, dtype=np.float32) * SCALE).reshape(NM, MCH).T
    )
    maps = []
    for c in range(NCORES):
        m = {"wt16": wt16, "xt16": xt16_all[c], "bias_img": bias_img}
        if P8:
            m["wt8"] = wt8
            m["wt8h"] = wt8h
            m["xt8"] = xt8_all[c]
        maps.append(m)
    return maps


def _assemble(results):
    out = np.empty((TOKENS, OUT), dtype=np.float32)
    inv = np.float32(1.0 / SCALE)
    for c, r in enumerate(results):
        out[c * TPC:(c + 1) * TPC] = r["outT"].reshape(OUT, TPC).T * inv
    return out


def kernel(x, weight_data, bias, block_rows, block_cols):
    nc = _get_nc()
    in_maps = _make_in_maps(x, weight_data, bias, block_rows, block_cols)
    res = run_bass_kernel_spmd(nc, in_maps, core_ids=list(range(NCORES)))
    return _assemble(res.results)
